# revision 1
# baseline (speedup 1.0000x reference)
"""DeepSeek-V3 MoE (T=4096, H=2048, E=32 top-8/32 grouped, I=1024, IS=2048)
on 8 trn2 NeuronCores — self-contained kernel.

Strategy (expert-parallel + token dispatch per the sharding hint):
- Routing (gate gemm + sigmoid + grouped top-k) runs on host in fp32: it is
  0.03%% of FLOPs, but expert SELECTION must match the fp32 reference exactly;
  the per-token combine weights ship to the device and fold into the phase-D
  output copy.
- Experts are load-balanced across cores: experts sorted by token count,
  rank-r octile assigned to slot r of each core, slot capacity = octile max
  (rounded to 16). This keeps the SPMD program identical across cores while
  minimizing padded compute (~4.2k routed token-slots/core vs 5.1k uniform).
- All gemms run bf16 on the PE (full rate, halves DMA traffic vs fp32).
- DMAs are batched into partition-contiguous transfers (host pre-lays-out
  weights/activations so each DMA moves 4-40KB per partition in one
  instruction): ~140 DMA instructions per iteration vs ~1400 naive, which
  keeps the SP sequencer (~800ns/DMA issue) off the critical path.
- The shared expert runs at full intermediate width over this core's T/8
  token slice (token-parallel: outputs disjoint, no collective needed).
- Host scatters the dispatched outputs back and adds shared slices.
"""
import contextlib
import numpy as np
import ml_dtypes

import concourse.bass as bass
import concourse.mybir as mybir
import concourse.tile as tile
from concourse import bacc

F32 = mybir.dt.float32
BF = mybir.dt.bfloat16
AF = mybir.ActivationFunctionType
BF_NP = ml_dtypes.bfloat16

TOP_K, N_GROUP, TOPK_GROUP, ROUTED_SCALE = 8, 8, 4, 2.5
T, H, E, I, IS = 4096, 2048, 32, 1024, 2048
N_CORES = 8
EL = E // N_CORES          # 4 expert slots per core
TSH = T // N_CORES         # 512-token shared slice per core
HT = H // 128              # 16
HT2 = HT // 2              # 8
IT = I // 128              # 8
IST = IS // 128            # 16


def host_routing(x, gate_w, e_bias):
    """fp32 numpy replica of reference _routing_weights -> dense [T, E]."""
    logits = (x @ gate_w.T).astype(np.float32)
    scores = (1.0 / (1.0 + np.exp(-logits.astype(np.float32)))).astype(np.float32)
    swb = scores + e_bias[None, :].astype(np.float32)
    t, e = swb.shape
    gsz = e // N_GROUP
    grp = swb.reshape(t, N_GROUP, gsz)
    top2 = np.sort(grp, axis=-1)[:, :, -2:]
    gscores = top2.sum(-1, dtype=np.float32)
    gidx = np.argsort(-gscores, axis=-1, kind="stable")[:, :TOPK_GROUP]
    gmask = np.zeros((t, N_GROUP), bool)
    np.put_along_axis(gmask, gidx, True, axis=1)
    emask = np.repeat(gmask, gsz, axis=1)
    masked = np.where(emask, swb, -np.inf)
    idx = np.argsort(-masked, axis=-1, kind="stable")[:, :TOP_K]
    w = np.take_along_axis(scores, idx, axis=1)
    w = (w / (w.sum(-1, keepdims=True) + 1e-20) * ROUTED_SCALE).astype(np.float32)
    wfull = np.zeros((t, e), np.float32)
    np.put_along_axis(wfull, idx, w, axis=1)
    return wfull


def make_plan(counts):
    """caps (per-slot capacities, SPMD-global) + assign[core][slot]=expert."""
    counts = np.asarray(counts)
    order = np.argsort(-counts, kind="stable")
    caps, assign = [], [[] for _ in range(N_CORES)]
    for r in range(E // N_CORES):
        octile = order[r * N_CORES : (r + 1) * N_CORES]
        cap = int(((int(counts[octile[0]]) + 7) // 8) * 8)
        caps.append(max(cap, 16))
        for m in range(N_CORES):
            assign[m].append(int(octile[m]))
    return tuple(caps), assign


def blocks_of(c):
    """Even split of capacity c (multiple of 16) into <=512 blocks (mult 16)."""
    nb = (c + 511) // 512
    n8 = c // 8
    sizes = [(n8 // nb + (1 if i < n8 % nb else 0)) * 8 for i in range(nb)]
    out, off = [], 0
    for s in sizes:
        out.append((off, s))
        off += s
    return out


def build_nc3(caps, tsh=TSH, repeat=1, mode="full"):
    dma_only, pe_only = mode == "dma", mode == "pe"
    totcap = sum(caps)
    offs = np.concatenate([[0], np.cumsum(caps)]).astype(int)
    max_c = max(max(caps), tsh)
    nc = bacc.Bacc("TRN2", target_bir_lowering=False)

    xs_d = nc.dram_tensor("xs", [128, HT * totcap], BF, kind="ExternalInput")
    colw_d = nc.dram_tensor("colw", [128, totcap], BF, kind="ExternalInput")
    w13_d = nc.dram_tensor("w13", [EL, IT, 128, HT * 256], BF,
                           kind="ExternalInput")
    w2_d = nc.dram_tensor("w2", [EL, HT2, 128, IT * 256], BF,
                          kind="ExternalInput")
    xsh_d = nc.dram_tensor("xsh", [128, HT * tsh], BF, kind="ExternalInput")
    sw13_d = nc.dram_tensor("sw13", [IST, 128, HT * 256], BF,
                            kind="ExternalInput")
    sw2_d = nc.dram_tensor("sw2", [HT2, 128, IST * 256], BF,
                           kind="ExternalInput")
    yd_d = nc.dram_tensor("yd", [HT2, 128, 2 * totcap], BF,
                          kind="ExternalOutput")
    ys_d = nc.dram_tensor("ys", [HT2, 128, 2 * tsh], BF,
                          kind="ExternalOutput")

    with tile.TileContext(nc) as tc:
        with (
            tc.tile_pool(name="xp", bufs=2) as xp,
            tc.tile_pool(name="cp", bufs=1) as cp,
            tc.tile_pool(name="wp", bufs=3) as wp,
            tc.tile_pool(name="w2p", bufs=4) as w2p,
            tc.tile_pool(name="hp", bufs=1) as hp,
            tc.tile_pool(name="sp", bufs=4) as sp,
            tc.tile_pool(name="yp", bufs=2) as yp,
            tc.tile_pool(name="ps", bufs=1, space="PSUM") as ps,
        ):
            static_w = {}
            if pe_only:
                w13st = wp.tile([128, HT * 256], BF, tag="w13", name="w13st")
                nc.sync.dma_start(w13st[:], w13_d[0, 0])
                w2st = w2p.tile([128, IST * 256], BF, tag="w2", name="w2st")
                nc.sync.dma_start(w2st[:], sw2_d[0])
                static_w = {"w13": w13st, "w2": w2st}
            hint = (mybir.EngineType.PE, mybir.EngineType.DVE,
                    mybir.EngineType.Activation, mybir.EngineType.SP)
            rep = (tc.For_i(0, repeat, 1, hint_engines=hint)
                   if repeat > 1 else contextlib.nullcontext())
            with rep:
                col_sb = None

                # ============ routed experts over dispatched tokens
                for j in range(EL):
                    c = caps[j]
                    off = int(offs[j])
                    blks = blocks_of(c)
                    if pe_only:
                        w13_0 = static_w["w13"]
                    else:
                        w13_0 = wp.tile([128, HT * 256], BF, tag="w13",
                                        name="w13_sb")
                        nc.sync.dma_start(w13_0[:], w13_d[j, 0])
                    x_tiles = []
                    xdma = (nc.sync, nc.sync, nc.sync, nc.sync)
                    for q in range(4):
                        xq = xp.tile([128, 4 * c], BF, tag=f"x{q}",
                                     name=f"x{q}")
                        xdma[q].dma_start(
                            xq[:],
                            xs_d[:, HT * off + q * 4 * c
                                 : HT * off + (q + 1) * 4 * c])
                        x_tiles.append(xq)
                    h_tiles = [hp.tile([128, max(caps)], BF, tag=f"h{ig}",
                                       name=f"h{ig}") for ig in range(IT)]
                    # ---- phase C: h = silu(w1@x) * (w3@x)
                    for ig in range(IT):
                        if ig == 0 or pe_only:
                            w13_sb = w13_0
                        else:
                            w13_sb = wp.tile([128, HT * 256], BF, tag="w13",
                                             name="w13_sb")
                            nc.sync.dma_start(w13_sb[:], w13_d[j, ig])
                        if dma_only:
                            continue
                        g_ps = [ps.tile([128, bs], F32, tag=f"g{b}",
                                        name=f"g_ps{b}")
                                for b, (_, bs) in enumerate(blks)]
                        u_ps = [ps.tile([128, bs], F32, tag=f"u{b}",
                                        name=f"u_ps{b}")
                                for b, (_, bs) in enumerate(blks)]
                        for h in range(HT):
                            w1ap = w13_sb[:, h * 256 : h * 256 + 128]
                            w3ap = w13_sb[:, h * 256 + 128 : h * 256 + 256]
                            xh = x_tiles[h // 4]
                            hq = h % 4
                            for b, (bo, bs) in enumerate(blks):
                                rhs = xh[:, hq * c + bo : hq * c + bo + bs]
                                nc.tensor.matmul(
                                    g_ps[b][:], w1ap, rhs,
                                    start=(h == 0), stop=(h == HT - 1))
                            for b, (bo, bs) in enumerate(blks):
                                rhs = xh[:, hq * c + bo : hq * c + bo + bs]
                                nc.tensor.matmul(
                                    u_ps[b][:], w3ap, rhs,
                                    start=(h == 0), stop=(h == HT - 1))
                        for b, (bo, bs) in enumerate(blks):
                            silu_sb = sp.tile([128, 512], BF, tag="silu",
                                              name="silu_sb")
                            nc.scalar.activation(silu_sb[:, :bs], g_ps[b][:],
                                                 AF.Silu)
                            nc.vector.tensor_mul(
                                h_tiles[ig][:, bo : bo + bs], u_ps[b][:],
                                silu_sb[:, :bs])

                    # ---- phase D: yd = (w2 @ h) * col
                    if col_sb is None:
                        col_sb = cp.tile([128, totcap], BF, tag="col",
                                         name="col_sb")
                        nc.sync.dma_start(col_sb[:], colw_d[:, :])
                    for hg in range(HT2):
                        if pe_only:
                            w2_sb = static_w["w2"]
                        else:
                            w2_sb = w2p.tile([128, IST * 256], BF, tag="w2",
                                             name="w2_sb")
                            nc.sync.dma_start(w2_sb[:, : IT * 256],
                                              w2_d[j, hg])
                        if dma_only:
                            nc.sync.dma_start(
                                yd_d[hg][:, 2 * off : 2 * off + 2 * c],
                                x_tiles[0][:, : 2 * c])
                            continue
                        yd_sb = yp.tile([128, 2 * max_c], BF, tag="ydst",
                                        name="yd_sb")
                        # i-outer so each w2 column block is loaded into the
                        # PE once per nblk chains instead of once per matmul
                        tagsets = (("o0", "o1", "g0"), ("g1", "g2", "u0"))
                        for hl in range(2):
                            o_list = [ps.tile([128, bs], F32,
                                              tag=tagsets[hl][b],
                                              name=f"o_ps{b}")
                                      for b, (_, bs) in enumerate(blks)]
                            for i in range(IT):
                                w2ap = w2_sb[:, i * 256 + hl * 128
                                             : i * 256 + hl * 128 + 128]
                                for b, (bo, bs) in enumerate(blks):
                                    nc.tensor.matmul(
                                        o_list[b][:], w2ap,
                                        h_tiles[i][:, bo : bo + bs],
                                        start=(i == 0), stop=(i == IT - 1))
                            for b, (bo, bs) in enumerate(blks):
                                nc.vector.tensor_mul(
                                    yd_sb[:, hl * c + bo : hl * c + bo + bs],
                                    o_list[b][:],
                                    col_sb[:, off + bo : off + bo + bs])
                        nc.sync.dma_start(
                            yd_d[hg][:, 2 * off : 2 * off + 2 * c],
                            yd_sb[:, : 2 * c])

                # ============ shared expert, full IS, this core's tsh tokens
                x_tiles = []
                xdma = (nc.sync, nc.sync, nc.sync, nc.sync)
                for q in range(4):
                    xq = xp.tile([128, 4 * tsh], BF, tag=f"x{q}",
                                 name=f"xsh{q}")
                    xdma[q].dma_start(
                        xq[:], xsh_d[:, q * 4 * tsh : (q + 1) * 4 * tsh])
                    x_tiles.append(xq)
                hs_tiles = [hp.tile([128, tsh], BF, tag=f"hs{ig}",
                                    name=f"hs{ig}") for ig in range(IST)]
                for ig in range(IST):
                    if pe_only:
                        w13_sb = static_w["w13"]
                    else:
                        w13_sb = wp.tile([128, HT * 256], BF, tag="w13",
                                         name="w13_sb")
                        nc.sync.dma_start(w13_sb[:], sw13_d[ig])
                    if dma_only:
                        continue
                    g_ps = ps.tile([128, tsh], F32, tag=f"g{ig % 2}",
                                   name="g_ps0")
                    u_ps = ps.tile([128, tsh], F32, tag=f"u{ig % 2}",
                                   name="u_ps0")
                    for h in range(HT):
                        rhs = x_tiles[h // 4][:, (h % 4) * tsh
                                              : (h % 4 + 1) * tsh]
                        nc.tensor.matmul(
                            g_ps[:], w13_sb[:, h * 256 : h * 256 + 128], rhs,
                            start=(h == 0), stop=(h == HT - 1))
                        nc.tensor.matmul(
                            u_ps[:], w13_sb[:, h * 256 + 128 : h * 256 + 256],
                            rhs, start=(h == 0), stop=(h == HT - 1))
                    silu_sb = sp.tile([128, 512], BF, tag="silu",
                                      name="silu_sb")
                    nc.scalar.activation(silu_sb[:, :tsh], g_ps[:], AF.Silu)
                    nc.vector.tensor_mul(hs_tiles[ig][:, :], u_ps[:],
                                         silu_sb[:, :tsh])
                for hg in range(HT2):
                    if pe_only:
                        w2_sb = static_w["w2"]
                    else:
                        w2_sb = w2p.tile([128, IST * 256], BF, tag="w2",
                                         name="w2_sb")
                        nc.sync.dma_start(w2_sb[:], sw2_d[hg])
                    if dma_only:
                        nc.sync.dma_start(ys_d[hg],
                                          x_tiles[0][:, : 2 * tsh])
                        continue
                    ys_sb = yp.tile([128, 2 * tsh], BF, tag="ydst",
                                    name="ys_sb")
                    for hl in range(2):
                        o_ps = ps.tile([128, tsh], F32, tag=f"o{hl}",
                                       name="o_ps")
                        for i in range(IST):
                            nc.tensor.matmul(
                                o_ps[:],
                                w2_sb[:, i * 256 + hl * 128
                                      : i * 256 + hl * 128 + 128],
                                hs_tiles[i][:, :],
                                start=(i == 0), stop=(i == IST - 1))
                        nc.vector.tensor_copy(
                            ys_sb[:, hl * tsh : (hl + 1) * tsh], o_ps[:])
                    nc.sync.dma_start(ys_d[hg], ys_sb[:])
    nc.compile()
    return nc


def _to_part_layout(xt, c):
    """[c, H] -> [128, HT*c] with [p, h*c+t] = xt[t, 128h+p]."""
    return np.ascontiguousarray(
        xt.T.reshape(HT, 128, c).transpose(1, 0, 2).reshape(128, HT * c))


def prep_weights(w1, w3, w2, sw1, sw3, sw2):
    w1 = np.asarray(w1, np.float32)
    w3 = np.asarray(w3, np.float32)
    w2 = np.asarray(w2, np.float32)
    w1r = w1.reshape(E, IT, 128, HT, 128).transpose(0, 1, 4, 3, 2)
    w3r = w3.reshape(E, IT, 128, HT, 128).transpose(0, 1, 4, 3, 2)
    w13 = np.ascontiguousarray(
        np.concatenate([w1r, w3r], -1).reshape(E, IT, 128, HT * 256)
    ).astype(BF_NP)
    w2r = w2.reshape(E, HT2, 2, 128, IT, 128).transpose(0, 1, 5, 4, 2, 3)
    w2l = np.ascontiguousarray(w2r.reshape(E, HT2, 128, IT * 256)).astype(BF_NP)

    sw1 = np.asarray(sw1, np.float32)
    sw3 = np.asarray(sw3, np.float32)
    sw2 = np.asarray(sw2, np.float32)
    s1r = sw1.reshape(IST, 128, HT, 128).transpose(0, 3, 2, 1)
    s3r = sw3.reshape(IST, 128, HT, 128).transpose(0, 3, 2, 1)
    sw13 = np.ascontiguousarray(
        np.concatenate([s1r, s3r], -1).reshape(IST, 128, HT * 256)
    ).astype(BF_NP)
    s2r = sw2.reshape(HT2, 2, 128, IST, 128).transpose(0, 4, 3, 1, 2)
    sw2l = np.ascontiguousarray(s2r.reshape(HT2, 128, IST * 256)).astype(BF_NP)
    return w13, w2l, sw13, sw2l


def prep_inputs3(hidden_states, gate_w, e_bias, w1, w3, w2, sw1, sw3, sw2,
                 caps=None, assign=None, wfull=None):
    x = np.asarray(hidden_states, np.float32)
    t_total = x.shape[0]
    tsh = t_total // N_CORES
    if wfull is None:
        wfull = host_routing(x, np.asarray(gate_w, np.float32),
                             np.asarray(e_bias, np.float32))
    if caps is None:
        caps, assign = make_plan((wfull != 0).sum(0))
    totcap = sum(caps)
    offs = np.concatenate([[0], np.cumsum(caps)]).astype(int)
    xb = x.astype(BF_NP)

    w13, w2l, sw13, sw2l = prep_weights(w1, w3, w2, sw1, sw3, sw2)

    in_maps, scat = [], []
    for m in range(N_CORES):
        slots = assign[m]
        xs = np.zeros((128, HT * totcap), BF_NP)
        col = np.zeros(totcap, np.float32)
        idxs = []
        for r, e in enumerate(slots):
            idx = np.nonzero(wfull[:, e])[0]
            assert len(idx) <= caps[r], (m, r, e, len(idx), caps[r])
            idxs.append(idx)
            pad = caps[r] - len(idx)
            tok = np.concatenate([idx, np.zeros(pad, np.int64)])
            xs[:, HT * offs[r] : HT * offs[r + 1]] = _to_part_layout(
                xb[tok], caps[r])
            col[offs[r] : offs[r] + len(idx)] = wfull[idx, e]
        colb = np.ascontiguousarray(
            np.broadcast_to(col.astype(BF_NP)[None, :], (128, totcap)))
        in_maps.append({
            "xs": xs,
            "colw": colb,
            "w13": np.ascontiguousarray(w13[slots]),
            "w2": np.ascontiguousarray(w2l[slots]),
            "xsh": _to_part_layout(xb[m * tsh : (m + 1) * tsh], tsh),
            "sw13": sw13, "sw2": sw2l,
        })
        scat.append(idxs)
    return in_maps, scat


def combine3(results, scat, caps, t_total=T):
    tsh = t_total // N_CORES
    offs = np.concatenate([[0], np.cumsum(caps)]).astype(int)
    accT = np.zeros((H, t_total), np.float32)
    for m in range(N_CORES):
        ys = np.asarray(results[m]["ys"], dtype=BF_NP).astype(np.float32)
        accT[:, m * tsh : (m + 1) * tsh] = (
            ys.reshape(HT2, 128, 2, tsh).transpose(0, 2, 1, 3)
            .reshape(H, tsh))
    for m in range(N_CORES):
        yd = np.asarray(results[m]["yd"], dtype=BF_NP).astype(np.float32)
        for r, idx in enumerate(scat[m]):
            c = caps[r]
            blk = (yd[:, :, 2 * offs[r] : 2 * offs[r] + 2 * c]
                   .reshape(HT2, 128, 2, c).transpose(0, 2, 1, 3)
                   .reshape(H, c))
            accT[:, idx] += blk[:, : len(idx)]
    return np.ascontiguousarray(accT.T)


_NC_CACHE = {}


def run3(inputs):
    from concourse.bass_utils import run_bass_kernel_spmd
    x = np.asarray(inputs["hidden_states"], np.float32)
    wfull = host_routing(x, np.asarray(inputs["gate_w"], np.float32),
                         np.asarray(inputs["e_bias"], np.float32))
    caps, assign = make_plan((wfull != 0).sum(0))
    key = (caps, x.shape[0])
    if key not in _NC_CACHE:
        _NC_CACHE[key] = build_nc3(caps, tsh=x.shape[0] // N_CORES)
    nc = _NC_CACHE[key]
    in_maps, scat = prep_inputs3(**inputs, caps=caps, assign=assign,
                                 wfull=wfull)
    res = run_bass_kernel_spmd(nc, in_maps, core_ids=list(range(N_CORES)))
    return combine3(res.results, scat, caps, t_total=x.shape[0]), res


def kernel(**inputs) -> np.ndarray:
    out, _ = run3(inputs)
    return np.asarray(out, np.float32)



# revision 3
# speedup vs baseline: 1.0150x; 1.0150x over previous
"""DeepSeek-V3 MoE (T=4096, H=2048, E=32 top-8/32 grouped, I=1024, IS=2048)
on 8 trn2 NeuronCores — self-contained kernel.

Strategy (expert-parallel + token dispatch per the sharding hint):
- Routing (gate gemm + sigmoid + grouped top-k) runs on host in fp32: it is
  0.03%% of FLOPs, but expert SELECTION must match the fp32 reference exactly;
  the per-token combine weights ship to the device and fold into the phase-D
  output copy.
- Experts are load-balanced across cores: experts sorted by token count,
  rank-r octile assigned to slot r of each core, slot capacity = octile max
  (rounded to 16). This keeps the SPMD program identical across cores while
  minimizing padded compute (~4.2k routed token-slots/core vs 5.1k uniform).
- All gemms run bf16 on the PE (full rate, halves DMA traffic vs fp32).
- DMAs are batched into partition-contiguous transfers (host pre-lays-out
  weights/activations so each DMA moves 4-40KB per partition in one
  instruction): ~140 DMA instructions per iteration vs ~1400 naive, which
  keeps the SP sequencer (~800ns/DMA issue) off the critical path.
- The shared expert runs at full intermediate width over this core's T/8
  token slice (token-parallel: outputs disjoint, no collective needed).
- Host scatters the dispatched outputs back and adds shared slices.
"""
import contextlib
import numpy as np
import ml_dtypes

import concourse.bass as bass
import concourse.mybir as mybir
import concourse.tile as tile
from concourse import bacc

F32 = mybir.dt.float32
BF = mybir.dt.bfloat16
AF = mybir.ActivationFunctionType
BF_NP = ml_dtypes.bfloat16

TOP_K, N_GROUP, TOPK_GROUP, ROUTED_SCALE = 8, 8, 4, 2.5
T, H, E, I, IS = 4096, 2048, 32, 1024, 2048
N_CORES = 8
EL = E // N_CORES          # 4 expert slots per core
TSH = T // N_CORES         # 512-token shared slice per core
HT = H // 128              # 16
HT2 = HT // 2              # 8
IT = I // 128              # 8
IST = IS // 128            # 16


def host_routing(x, gate_w, e_bias):
    """fp32 numpy replica of reference _routing_weights -> dense [T, E]."""
    logits = (x @ gate_w.T).astype(np.float32)
    scores = (1.0 / (1.0 + np.exp(-logits.astype(np.float32)))).astype(np.float32)
    swb = scores + e_bias[None, :].astype(np.float32)
    t, e = swb.shape
    gsz = e // N_GROUP
    grp = swb.reshape(t, N_GROUP, gsz)
    top2 = np.sort(grp, axis=-1)[:, :, -2:]
    gscores = top2.sum(-1, dtype=np.float32)
    gidx = np.argsort(-gscores, axis=-1, kind="stable")[:, :TOPK_GROUP]
    gmask = np.zeros((t, N_GROUP), bool)
    np.put_along_axis(gmask, gidx, True, axis=1)
    emask = np.repeat(gmask, gsz, axis=1)
    masked = np.where(emask, swb, -np.inf)
    idx = np.argsort(-masked, axis=-1, kind="stable")[:, :TOP_K]
    w = np.take_along_axis(scores, idx, axis=1)
    w = (w / (w.sum(-1, keepdims=True) + 1e-20) * ROUTED_SCALE).astype(np.float32)
    wfull = np.zeros((t, e), np.float32)
    np.put_along_axis(wfull, idx, w, axis=1)
    return wfull


def make_plan(counts):
    """caps (per-slot capacities, SPMD-global) + assign[core][slot]=expert."""
    counts = np.asarray(counts)
    order = np.argsort(-counts, kind="stable")
    caps, assign = [], [[] for _ in range(N_CORES)]
    for r in range(E // N_CORES):
        octile = order[r * N_CORES : (r + 1) * N_CORES]
        cap = int(((int(counts[octile[0]]) + 7) // 8) * 8)
        caps.append(max(cap, 16))
        for m in range(N_CORES):
            assign[m].append(int(octile[m]))
    return tuple(caps), assign


def blocks_of(c):
    """Even split of capacity c (multiple of 16) into <=512 blocks (mult 16)."""
    nb = (c + 511) // 512
    n8 = c // 8
    sizes = [(n8 // nb + (1 if i < n8 % nb else 0)) * 8 for i in range(nb)]
    out, off = [], 0
    for s in sizes:
        out.append((off, s))
        off += s
    return out


def build_nc3(caps, tsh=TSH, repeat=1, mode="full"):
    dma_only, pe_only = mode == "dma", mode == "pe"
    totcap = sum(caps)
    offs = np.concatenate([[0], np.cumsum(caps)]).astype(int)
    max_c = max(max(caps), tsh)
    nc = bacc.Bacc("TRN2", target_bir_lowering=False)

    xs_d = nc.dram_tensor("xs", [128, HT * totcap], BF, kind="ExternalInput")
    colw_d = nc.dram_tensor("colw", [128, totcap], BF, kind="ExternalInput")
    w13_d = nc.dram_tensor("w13", [EL, IT, 128, HT * 256], BF,
                           kind="ExternalInput")
    w2_d = nc.dram_tensor("w2", [EL, HT2, 128, IT * 256], BF,
                          kind="ExternalInput")
    xsh_d = nc.dram_tensor("xsh", [128, HT * tsh], BF, kind="ExternalInput")
    sw13_d = nc.dram_tensor("sw13", [IST, 128, HT * 256], BF,
                            kind="ExternalInput")
    sw2_d = nc.dram_tensor("sw2", [HT2, 128, IST * 256], BF,
                           kind="ExternalInput")
    yd_d = nc.dram_tensor("yd", [HT2, 128, 2 * totcap], BF,
                          kind="ExternalOutput")
    ys_d = nc.dram_tensor("ys", [HT2, 128, 2 * tsh], BF,
                          kind="ExternalOutput")

    with tile.TileContext(nc) as tc:
        with (
            tc.tile_pool(name="xp", bufs=2) as xp,
            tc.tile_pool(name="cp", bufs=1) as cp,
            tc.tile_pool(name="wp", bufs=3) as wp,
            tc.tile_pool(name="w2p", bufs=4) as w2p,
            tc.tile_pool(name="hp", bufs=1) as hp,
            tc.tile_pool(name="sp", bufs=4) as sp,
            tc.tile_pool(name="yp", bufs=2) as yp,
            tc.tile_pool(name="ps", bufs=1, space="PSUM") as ps,
        ):
            static_w = {}
            if pe_only:
                w13st = wp.tile([128, HT * 256], BF, tag="w13", name="w13st")
                nc.sync.dma_start(w13st[:], w13_d[0, 0])
                w2st = w2p.tile([128, IST * 256], BF, tag="w2", name="w2st")
                nc.sync.dma_start(w2st[:], sw2_d[0])
                static_w = {"w13": w13st, "w2": w2st}
            hint = (mybir.EngineType.PE, mybir.EngineType.DVE,
                    mybir.EngineType.Activation, mybir.EngineType.SP)
            rep = (tc.For_i(0, repeat, 1, hint_engines=hint)
                   if repeat > 1 else contextlib.nullcontext())
            with rep:
                col_sb = None

                # ============ routed experts over dispatched tokens
                for j in range(EL):
                    c = caps[j]
                    off = int(offs[j])
                    blks = blocks_of(c)
                    if pe_only:
                        w13_0 = static_w["w13"]
                    else:
                        w13_0 = wp.tile([128, HT * 256], BF, tag="w13",
                                        name="w13_sb")
                        nc.sync.dma_start(w13_0[:], w13_d[j, 0])
                    x_tiles = []
                    xdma = (nc.sync, nc.sync, nc.sync, nc.sync)
                    for q in range(4):
                        xq = xp.tile([128, 4 * c], BF, tag=f"x{q}",
                                     name=f"x{q}")
                        xdma[q].dma_start(
                            xq[:],
                            xs_d[:, HT * off + q * 4 * c
                                 : HT * off + (q + 1) * 4 * c])
                        x_tiles.append(xq)
                    h_tiles = [hp.tile([128, max(caps)], BF, tag=f"h{ig}",
                                       name=f"h{ig}") for ig in range(IT)]
                    # ---- phase C: h = silu(w1@x) * (w3@x)
                    # Per PSUM bank, run the full 16-step accumulation chain
                    # back-to-back: bank switches between consecutive matmuls
                    # cost ~33ns/MM on HW (PSUM-queue cycling), while weight
                    # switches are free (hidden by the background weight
                    # buffer when N/2.4GHz exceeds the ~114ns LDWEIGHTS).
                    for ig in range(IT):
                        if ig == 0 or pe_only:
                            w13_sb = w13_0
                        else:
                            w13_sb = wp.tile([128, HT * 256], BF, tag="w13",
                                             name="w13_sb")
                            nc.sync.dma_start(w13_sb[:], w13_d[j, ig])
                        if dma_only:
                            continue
                        for b, (bo, bs) in enumerate(blks):
                            g_ps = ps.tile([128, bs], F32, tag=f"g{b}",
                                           name=f"g_ps{b}")
                            u_ps = ps.tile([128, bs], F32, tag=f"u{b}",
                                           name=f"u_ps{b}")
                            for h in range(HT):
                                rhs = x_tiles[h // 4][
                                    :, (h % 4) * c + bo
                                    : (h % 4) * c + bo + bs]
                                nc.tensor.matmul(
                                    g_ps[:],
                                    w13_sb[:, h * 256 : h * 256 + 128], rhs,
                                    start=(h == 0), stop=(h == HT - 1))
                            silu_sb = sp.tile([128, 512], BF, tag="silu",
                                              name="silu_sb")
                            nc.scalar.activation(silu_sb[:, :bs], g_ps[:],
                                                 AF.Silu)
                            for h in range(HT):
                                rhs = x_tiles[h // 4][
                                    :, (h % 4) * c + bo
                                    : (h % 4) * c + bo + bs]
                                nc.tensor.matmul(
                                    u_ps[:],
                                    w13_sb[:, h * 256 + 128
                                           : h * 256 + 256], rhs,
                                    start=(h == 0), stop=(h == HT - 1))
                            nc.vector.tensor_mul(
                                h_tiles[ig][:, bo : bo + bs], u_ps[:],
                                silu_sb[:, :bs])

                    # ---- phase D: yd = (w2 @ h) * col
                    if col_sb is None:
                        col_sb = cp.tile([128, totcap], BF, tag="col",
                                         name="col_sb")
                        nc.sync.dma_start(col_sb[:], colw_d[:, :])
                    for hg in range(HT2):
                        if pe_only:
                            w2_sb = static_w["w2"]
                        else:
                            w2_sb = w2p.tile([128, IST * 256], BF, tag="w2",
                                             name="w2_sb")
                            nc.sync.dma_start(w2_sb[:, : IT * 256],
                                              w2_d[j, hg])
                        if dma_only:
                            nc.sync.dma_start(
                                yd_d[hg][:, 2 * off : 2 * off + 2 * c],
                                x_tiles[0][:, : 2 * c])
                            continue
                        yd_sb = yp.tile([128, 2 * max_c], BF, tag="ydst",
                                        name="yd_sb")
                        tagsets = (("o0", "o1", "g0"), ("g1", "g2", "u0"))
                        for hl in range(2):
                            for b, (bo, bs) in enumerate(blks):
                                o_ps = ps.tile([128, bs], F32,
                                               tag=tagsets[hl][b],
                                               name=f"o_ps{b}")
                                for i in range(IT):
                                    nc.tensor.matmul(
                                        o_ps[:],
                                        w2_sb[:, i * 256 + hl * 128
                                              : i * 256 + hl * 128 + 128],
                                        h_tiles[i][:, bo : bo + bs],
                                        start=(i == 0), stop=(i == IT - 1))
                                nc.vector.tensor_mul(
                                    yd_sb[:, hl * c + bo : hl * c + bo + bs],
                                    o_ps[:],
                                    col_sb[:, off + bo : off + bo + bs])
                        nc.sync.dma_start(
                            yd_d[hg][:, 2 * off : 2 * off + 2 * c],
                            yd_sb[:, : 2 * c])

                # ============ shared expert, full IS, this core's tsh tokens
                x_tiles = []
                xdma = (nc.sync, nc.sync, nc.sync, nc.sync)
                for q in range(4):
                    xq = xp.tile([128, 4 * tsh], BF, tag=f"x{q}",
                                 name=f"xsh{q}")
                    xdma[q].dma_start(
                        xq[:], xsh_d[:, q * 4 * tsh : (q + 1) * 4 * tsh])
                    x_tiles.append(xq)
                hs_tiles = [hp.tile([128, tsh], BF, tag=f"hs{ig}",
                                    name=f"hs{ig}") for ig in range(IST)]
                for ig in range(IST):
                    if pe_only:
                        w13_sb = static_w["w13"]
                    else:
                        w13_sb = wp.tile([128, HT * 256], BF, tag="w13",
                                         name="w13_sb")
                        nc.sync.dma_start(w13_sb[:], sw13_d[ig])
                    if dma_only:
                        continue
                    g_ps = ps.tile([128, tsh], F32, tag=f"g{ig % 2}",
                                   name="g_ps0")
                    u_ps = ps.tile([128, tsh], F32, tag=f"u{ig % 2}",
                                   name="u_ps0")
                    for h in range(HT):
                        rhs = x_tiles[h // 4][:, (h % 4) * tsh
                                              : (h % 4 + 1) * tsh]
                        nc.tensor.matmul(
                            g_ps[:], w13_sb[:, h * 256 : h * 256 + 128], rhs,
                            start=(h == 0), stop=(h == HT - 1))
                    silu_sb = sp.tile([128, 512], BF, tag="silu",
                                      name="silu_sb")
                    nc.scalar.activation(silu_sb[:, :tsh], g_ps[:], AF.Silu)
                    for h in range(HT):
                        rhs = x_tiles[h // 4][:, (h % 4) * tsh
                                              : (h % 4 + 1) * tsh]
                        nc.tensor.matmul(
                            u_ps[:], w13_sb[:, h * 256 + 128 : h * 256 + 256],
                            rhs, start=(h == 0), stop=(h == HT - 1))
                    nc.vector.tensor_mul(hs_tiles[ig][:, :], u_ps[:],
                                         silu_sb[:, :tsh])
                for hg in range(HT2):
                    if pe_only:
                        w2_sb = static_w["w2"]
                    else:
                        w2_sb = w2p.tile([128, IST * 256], BF, tag="w2",
                                         name="w2_sb")
                        nc.sync.dma_start(w2_sb[:], sw2_d[hg])
                    if dma_only:
                        nc.sync.dma_start(ys_d[hg],
                                          x_tiles[0][:, : 2 * tsh])
                        continue
                    ys_sb = yp.tile([128, 2 * tsh], BF, tag="ydst",
                                    name="ys_sb")
                    for hl in range(2):
                        o_ps = ps.tile([128, tsh], F32, tag=f"o{hl}",
                                       name="o_ps")
                        for i in range(IST):
                            nc.tensor.matmul(
                                o_ps[:],
                                w2_sb[:, i * 256 + hl * 128
                                      : i * 256 + hl * 128 + 128],
                                hs_tiles[i][:, :],
                                start=(i == 0), stop=(i == IST - 1))
                        nc.vector.tensor_copy(
                            ys_sb[:, hl * tsh : (hl + 1) * tsh], o_ps[:])
                    nc.sync.dma_start(ys_d[hg], ys_sb[:])
    nc.compile()
    return nc


def _to_part_layout(xt, c):
    """[c, H] -> [128, HT*c] with [p, h*c+t] = xt[t, 128h+p]."""
    return np.ascontiguousarray(
        xt.T.reshape(HT, 128, c).transpose(1, 0, 2).reshape(128, HT * c))


def prep_weights(w1, w3, w2, sw1, sw3, sw2):
    w1 = np.asarray(w1, np.float32)
    w3 = np.asarray(w3, np.float32)
    w2 = np.asarray(w2, np.float32)
    w1r = w1.reshape(E, IT, 128, HT, 128).transpose(0, 1, 4, 3, 2)
    w3r = w3.reshape(E, IT, 128, HT, 128).transpose(0, 1, 4, 3, 2)
    w13 = np.ascontiguousarray(
        np.concatenate([w1r, w3r], -1).reshape(E, IT, 128, HT * 256)
    ).astype(BF_NP)
    w2r = w2.reshape(E, HT2, 2, 128, IT, 128).transpose(0, 1, 5, 4, 2, 3)
    w2l = np.ascontiguousarray(w2r.reshape(E, HT2, 128, IT * 256)).astype(BF_NP)

    sw1 = np.asarray(sw1, np.float32)
    sw3 = np.asarray(sw3, np.float32)
    sw2 = np.asarray(sw2, np.float32)
    s1r = sw1.reshape(IST, 128, HT, 128).transpose(0, 3, 2, 1)
    s3r = sw3.reshape(IST, 128, HT, 128).transpose(0, 3, 2, 1)
    sw13 = np.ascontiguousarray(
        np.concatenate([s1r, s3r], -1).reshape(IST, 128, HT * 256)
    ).astype(BF_NP)
    s2r = sw2.reshape(HT2, 2, 128, IST, 128).transpose(0, 4, 3, 1, 2)
    sw2l = np.ascontiguousarray(s2r.reshape(HT2, 128, IST * 256)).astype(BF_NP)
    return w13, w2l, sw13, sw2l


def prep_inputs3(hidden_states, gate_w, e_bias, w1, w3, w2, sw1, sw3, sw2,
                 caps=None, assign=None, wfull=None):
    x = np.asarray(hidden_states, np.float32)
    t_total = x.shape[0]
    tsh = t_total // N_CORES
    if wfull is None:
        wfull = host_routing(x, np.asarray(gate_w, np.float32),
                             np.asarray(e_bias, np.float32))
    if caps is None:
        caps, assign = make_plan((wfull != 0).sum(0))
    totcap = sum(caps)
    offs = np.concatenate([[0], np.cumsum(caps)]).astype(int)
    xb = x.astype(BF_NP)

    w13, w2l, sw13, sw2l = prep_weights(w1, w3, w2, sw1, sw3, sw2)

    in_maps, scat = [], []
    for m in range(N_CORES):
        slots = assign[m]
        xs = np.zeros((128, HT * totcap), BF_NP)
        col = np.zeros(totcap, np.float32)
        idxs = []
        for r, e in enumerate(slots):
            idx = np.nonzero(wfull[:, e])[0]
            assert len(idx) <= caps[r], (m, r, e, len(idx), caps[r])
            idxs.append(idx)
            pad = caps[r] - len(idx)
            tok = np.concatenate([idx, np.zeros(pad, np.int64)])
            xs[:, HT * offs[r] : HT * offs[r + 1]] = _to_part_layout(
                xb[tok], caps[r])
            col[offs[r] : offs[r] + len(idx)] = wfull[idx, e]
        colb = np.ascontiguousarray(
            np.broadcast_to(col.astype(BF_NP)[None, :], (128, totcap)))
        in_maps.append({
            "xs": xs,
            "colw": colb,
            "w13": np.ascontiguousarray(w13[slots]),
            "w2": np.ascontiguousarray(w2l[slots]),
            "xsh": _to_part_layout(xb[m * tsh : (m + 1) * tsh], tsh),
            "sw13": sw13, "sw2": sw2l,
        })
        scat.append(idxs)
    return in_maps, scat


def combine3(results, scat, caps, t_total=T):
    tsh = t_total // N_CORES
    offs = np.concatenate([[0], np.cumsum(caps)]).astype(int)
    accT = np.zeros((H, t_total), np.float32)
    for m in range(N_CORES):
        ys = np.asarray(results[m]["ys"], dtype=BF_NP).astype(np.float32)
        accT[:, m * tsh : (m + 1) * tsh] = (
            ys.reshape(HT2, 128, 2, tsh).transpose(0, 2, 1, 3)
            .reshape(H, tsh))
    for m in range(N_CORES):
        yd = np.asarray(results[m]["yd"], dtype=BF_NP).astype(np.float32)
        for r, idx in enumerate(scat[m]):
            c = caps[r]
            blk = (yd[:, :, 2 * offs[r] : 2 * offs[r] + 2 * c]
                   .reshape(HT2, 128, 2, c).transpose(0, 2, 1, 3)
                   .reshape(H, c))
            accT[:, idx] += blk[:, : len(idx)]
    return np.ascontiguousarray(accT.T)


_NC_CACHE = {}


def run3(inputs):
    from concourse.bass_utils import run_bass_kernel_spmd
    x = np.asarray(inputs["hidden_states"], np.float32)
    wfull = host_routing(x, np.asarray(inputs["gate_w"], np.float32),
                         np.asarray(inputs["e_bias"], np.float32))
    caps, assign = make_plan((wfull != 0).sum(0))
    key = (caps, x.shape[0])
    if key not in _NC_CACHE:
        _NC_CACHE[key] = build_nc3(caps, tsh=x.shape[0] // N_CORES)
    nc = _NC_CACHE[key]
    in_maps, scat = prep_inputs3(**inputs, caps=caps, assign=assign,
                                 wfull=wfull)
    res = run_bass_kernel_spmd(nc, in_maps, core_ids=list(range(N_CORES)))
    return combine3(res.results, scat, caps, t_total=x.shape[0]), res


def kernel(**inputs) -> np.ndarray:
    out, _ = run3(inputs)
    return np.asarray(out, np.float32)

